# revision 4
# baseline (speedup 1.0000x reference)
"""Trainium2 kernel for nn_NetworkFull_9758165696599 (PointNet2 cage deformer).

Sharding: data-parallel over batch B=16 across 8 NeuronCores (2 samples/core,
source+target clouds = 4 encoder runs per core); the small conv/MLP weights
are replicated to every core. All dense compute (shared conv MLPs + relu,
max-pools, tanh post-convs, merger and both deformation MLPs) runs on-device
via Bass/Tile SPMD programs through run_bass_kernel_spmd. The tiny irregular
index computations (FPS, ball-query first-32 selection, gathers) run on host
in bit-exact float32 numpy between the three device stages, because this
container's walrus build rejects the GPSIMD gather/scatter ISA ops needed to
do them on-device ("ISA wrong length").
"""
import sys
import numpy as np
from contextlib import ExitStack

sys.path.insert(0, "/opt/trn_rl_repo")

import concourse.bass as bass  # noqa: E402
import concourse.mybir as mybir  # noqa: E402
import concourse.tile as tile  # noqa: E402
from concourse.bass_utils import run_bass_kernel_spmd  # noqa: E402
from bass_rust import ScopedClock, VectorClock  # noqa: E402

F32 = mybir.dt.float32
AF = mybir.ActivationFunctionType
ALU = mybir.AluOpType
AX = mybir.AxisListType

B, N, P = 16, 2048, 42
NPOINTS = [512, 128, 32]
RADII = [0.2, 0.4, 0.8]
NSAMPLE = 32
CH = [[3, 24, 48, 96], [99, 48, 96, 192], [195, 96, 192, 256]]
NCORES = 8
BPC = B // NCORES
NRUNS = 2 * BPC


# --- Tile tail-drain patch (walrus here rejects multi-wait SP TPB_CTRL) ----
def _patched_drain_and_barrier(self, tick_clock, wait_clock):
    nc = self.nc
    ticks = list(tick_clock.global_clock)
    for p in range(len(ticks)):
        if ticks[p] > 0:
            vec = [ticks[i] if i == p else 0 for i in range(len(ticks))]
            nop_inst = nc.sync.nop()
            wait_clock.add_sem_waits(nop_inst.ins, ScopedClock({None: VectorClock(vec)}))
    nc.sync.drain()
    nc.all_engine_barrier()
    assert self.sems is not None
    popped = nc._tile_sem_poison_stack.pop()
    assert popped is self._sem_poison
    nc.clear_and_free_semaphores(list(self.sems.allocated().values()))
    nc.all_engine_barrier()


tile.TileContext._drain_and_barrier = _patched_drain_and_barrier


def _split_waits(nc, limit=1):
    """This walrus build rejects instructions with more than ~1 sync wait;
    move excess waits onto same-engine NoOps inserted just before."""
    seq = [0]
    for bbw in nc.bb_map.values():
        bb = bbw.bb
        out, changed = [], False
        for inst in bb.instructions:
            si = inst.sync_info
            waits = list(si.on_wait) if si and si.on_wait else []
            if len(waits) > limit:
                changed = True
                extra, keep = waits[:-limit], waits[-limit:]
                for i in range(0, len(extra), limit):
                    seq[0] += 1
                    nop = mybir.InstNoOp(name=f"nopw{seq[0]}", ins=[], outs=[])
                    nop.engine = inst.engine
                    nop.sync_info = mybir.SyncInfo(on_wait=extra[i:i + limit],
                                                   on_update=[])
                    out.append(nop)
                inst.sync_info = mybir.SyncInfo(
                    on_wait=keep,
                    on_update=list(si.on_update) if si.on_update else [])
            out.append(inst)
        if changed:
            bb.instructions = out


# --- host-side exact index computations (f32, same op order as reference) --
def _fps(xyz, npoint):
    Bq, Nq, _ = xyz.shape
    dist = np.full((Bq, Nq), 1e10, np.float32)
    far = np.zeros(Bq, np.int64)
    idx = np.zeros((Bq, npoint), np.int64)
    ar = np.arange(Bq)
    for s in range(npoint):
        idx[:, s] = far
        c = xyz[ar, far]
        dx = xyz[:, :, 0] - c[:, None, 0]
        dy = xyz[:, :, 1] - c[:, None, 1]
        dz = xyz[:, :, 2] - c[:, None, 2]
        d = (dx * dx + dy * dy) + dz * dz
        dist = np.minimum(dist, d)
        far = np.argmax(dist, axis=-1)
    return idx


def _group(xyz, new_xyz, radius, nsample):
    dx = new_xyz[:, :, None, 0] - xyz[:, None, :, 0]
    dy = new_xyz[:, :, None, 1] - xyz[:, None, :, 1]
    dz = new_xyz[:, :, None, 2] - xyz[:, None, :, 2]
    d2 = (dx * dx + dy * dy) + dz * dz
    mask = d2 < np.float32(radius * radius)
    key = np.where(mask, 0, 1).astype(np.int64)
    grp = np.argsort(key, axis=-1, kind="stable")[..., :nsample]
    cnt = mask.sum(-1, keepdims=True)
    valid = np.arange(nsample)[None, None, :] < cnt
    return np.where(valid, grp, grp[..., :1])


# --- device program builders -----------------------------------------------
def _blocks(n, bs=128):
    return [(i, min(i + bs, n)) for i in range(0, n, bs)]


class BMat:
    def __init__(self, pool, nc, K, M, tag, dram=None, dma_engine=None):
        self.K, self.M = K, M
        self.bl = _blocks(K)
        self.t = []
        for i, (a, b) in enumerate(self.bl):
            til = pool.tile([b - a, M], F32, tag=f"{tag}_{i}")
            self.t.append(til)
            if dram is not None:
                (dma_engine or nc.sync).dma_start(til[:], dram[a:b])

    def blk(self, i):
        return self.t[i]


def _matmul_block(nc, pp, out_fn, w, rhs, ncols, col_off, ptag="psA"):
    for mi, (m0, m1) in enumerate(_blocks(w.M)):
        ps = pp.tile([m1 - m0, ncols], F32, tag=ptag)
        nk = len(w.bl)
        for ki in range(nk):
            nc.tensor.matmul(
                ps[:], w.blk(ki)[:, m0:m1],
                rhs.blk(ki)[:, col_off:col_off + ncols],
                start=(ki == 0), stop=(ki == nk - 1))
        out_fn(mi, m0, m1, ps)


def _build_enc_program(lvl):
    S = NPOINTS[lvl]
    cin, c1, c2, c3 = CH[lvl]
    K1 = 16 * ((cin + 15) // 16)
    COLS = S * NSAMPLE
    CHK = min(512, COLS)
    NCH = COLS // CHK
    sg = CHK // NSAMPLE

    nc = bass.Bass()
    gin = nc.declare_dram_parameter("gin", [NRUNS, K1, COLS], F32, isOutput=False)
    dw = {}
    for nm, shp in (("w1", [K1, c1]), ("w2", [c1, c2]), ("w3", [c2, c3]),
                    ("pw", [c3, 256]), ("b1", [c1, 1]), ("b2", [c2, 1]),
                    ("b3", [c3, 1]), ("pb", [256, 1])):
        dw[nm] = nc.declare_dram_parameter(nm, shp, F32, isOutput=False)
    feats_o = nc.declare_dram_parameter("feats", [NRUNS, c3, S], F32, isOutput=True)
    code_o = nc.declare_dram_parameter("code", [NRUNS, 256], F32, isOutput=True)

    ctx = ExitStack()
    tc = ctx.enter_context(tile.TileContext(nc))
    wp = ctx.enter_context(tc.tile_pool(name="w", bufs=1))
    ip = ctx.enter_context(tc.tile_pool(name="in", bufs=2))
    ap_ = ctx.enter_context(tc.tile_pool(name="act", bufs=2))
    op_ = ctx.enter_context(tc.tile_pool(name="out", bufs=2))
    pp = ctx.enter_context(tc.tile_pool(name="ps", bufs=2, space="PSUM"))

    w1 = BMat(wp, nc, K1, c1, "w1", dw["w1"])
    w2 = BMat(wp, nc, c1, c2, "w2", dw["w2"])
    w3 = BMat(wp, nc, c2, c3, "w3", dw["w3"])
    pw = BMat(wp, nc, c3, 256, "pw", dw["pw"])
    b1 = BMat(wp, nc, c1, 1, "b1", dw["b1"])
    b2 = BMat(wp, nc, c2, 1, "b2", dw["b2"])
    b3 = BMat(wp, nc, c3, 1, "b3", dw["b3"])
    pb = BMat(wp, nc, 256, 1, "pb", dw["pb"])

    for r in range(NRUNS):
        tin = BMat(ip, nc, K1, COLS, "tin", gin[r])
        pooled = BMat(op_, nc, c3, S, "pooled")
        fe = BMat(op_, nc, c3, S, "fe")
        for i in range(NCH):
            a1c = BMat(ap_, nc, c1, CHK, "a1c")
            a2c = BMat(ap_, nc, c2, CHK, "a2c")

            def into_a1(mi, m0, m1, ps):
                nc.scalar.activation(a1c.blk(mi)[:], ps[:], AF.Relu,
                                     bias=b1.blk(mi)[:, 0:1])

            _matmul_block(nc, pp, into_a1, w1, tin, CHK, i * CHK)

            def into_a2(mi, m0, m1, ps):
                nc.scalar.activation(a2c.blk(mi)[:], ps[:], AF.Relu,
                                     bias=b2.blk(mi)[:, 0:1])

            _matmul_block(nc, pp, into_a2, w2, a1c, CHK, 0)

            def into_pool(mi, m0, m1, ps, i=i):
                nc.vector.tensor_reduce(
                    out=pooled.blk(mi)[:, i * sg:(i + 1) * sg],
                    in_=ps[:].rearrange("c (s k) -> c s k", k=NSAMPLE),
                    axis=AX.X, op=ALU.max)

            _matmul_block(nc, pp, into_pool, w3, a2c, CHK, 0)
        for mi, (a, b_) in enumerate(fe.bl):
            nc.scalar.activation(fe.blk(mi)[:], pooled.blk(mi)[:], AF.Relu,
                                 bias=b3.blk(mi)[:, 0:1])
            nc.sync.dma_start(feats_o[r, a:b_], fe.blk(mi)[:])
        cd = op_.tile([128, 2], F32, tag="cd")

        def into_code(mi, m0, m1, ps):
            th = op_.tile([m1 - m0, S], F32, tag=f"th{mi}")
            nc.scalar.activation(th[:], ps[:], AF.Tanh, bias=pb.blk(mi)[:, 0:1])
            nc.vector.tensor_reduce(out=cd[:, mi:mi + 1], in_=th[:], axis=AX.X,
                                    op=ALU.max)

        _matmul_block(nc, pp, into_code, pw, fe, S, 0)
        nc.sync.dma_start(code_o[r, 0:128].rearrange("(c u) -> c u", u=1), cd[:, 0:1])
        nc.sync.dma_start(code_o[r, 128:256].rearrange("(c u) -> c u", u=1), cd[:, 1:2])
    ctx.close()
    _split_waits(nc)
    return nc


def _build_head_program():
    nc = bass.Bass()
    codes = nc.declare_dram_parameter("codes", [NRUNS, 512], F32, isOutput=False)
    dr = {}
    for nm, shp in (("wnc", [512, 512]), ("wnc1", [512, 256]), ("wnc2", [256, 128]),
                    ("wnd", [1024, 512]), ("wnd1", [512, 256]), ("wnd2", [256, 128]),
                    ("wmg", [1024, 1024]), ("bnc", [512, 3]), ("bnd", [512, 3]),
                    ("bmg", [1024, 1]), ("tmpl", [128, 1])):
        dr[nm] = nc.declare_dram_parameter(nm, shp, F32, isOutput=False)
    outv = nc.declare_dram_parameter("outv", [2, BPC, 128], F32, isOutput=True)

    ctx = ExitStack()
    tc = ctx.enter_context(tile.TileContext(nc))
    wp = ctx.enter_context(tc.tile_pool(name="w", bufs=1))
    sp = ctx.enter_context(tc.tile_pool(name="s", bufs=1))
    pp = ctx.enter_context(tc.tile_pool(name="p", bufs=2, space="PSUM"))

    W = {nm: BMat(wp, nc, dr[nm].shape[0], dr[nm].shape[1], nm, dr[nm]) for nm in dr}
    scode = BMat(sp, nc, 512, BPC, "scode")
    tcode = BMat(sp, nc, 512, BPC, "tcode")
    for b_ in range(BPC):
        for ki, (a, bb) in enumerate(_blocks(512)):
            nc.sync.dma_start(scode.blk(ki)[:, b_:b_ + 1],
                              codes[b_, a:bb].rearrange("(c u) -> c u", u=1))
            nc.sync.dma_start(tcode.blk(ki)[:, b_:b_ + 1],
                              codes[BPC + b_, a:bb].rearrange("(c u) -> c u", u=1))

    def lrelu_into(dst, bmat, bcol, tag):
        def fn(mi, m0, m1, ps):
            t1 = sp.tile([m1 - m0, BPC], F32, tag=f"t1{tag}")
            nc.vector.tensor_scalar(out=t1[:], in0=ps[:],
                                    scalar1=bmat.blk(m0 // 128)[m0 % 128:m0 % 128 + (m1 - m0), bcol:bcol + 1],
                                    scalar2=None, op0=ALU.add)
            t2 = sp.tile([m1 - m0, BPC], F32, tag=f"t2{tag}")
            nc.vector.tensor_scalar_mul(t2[:], t1[:], 0.01)
            nc.vector.tensor_tensor(out=dst.blk(mi)[:], in0=t1[:], in1=t2[:], op=ALU.max)
        return fn

    def mlp3(inp, w0, w1, w2, bm, tag):
        h1 = BMat(sp, nc, 512, BPC, f"h1{tag}")
        _matmul_block(nc, pp, lrelu_into(h1, W[bm], 0, tag + "0"), W[w0], inp, BPC, 0)
        h2 = BMat(sp, nc, 256, BPC, f"h2{tag}")

        def l2fn(mi, m0, m1, ps):
            t1 = sp.tile([m1 - m0, BPC], F32, tag=f"t1{tag}b")
            nc.vector.tensor_scalar(out=t1[:], in0=ps[:],
                                    scalar1=W[bm].blk(mi)[0:(m1 - m0), 1:2],
                                    scalar2=None, op0=ALU.add)
            t2 = sp.tile([m1 - m0, BPC], F32, tag=f"t2{tag}b")
            nc.vector.tensor_scalar_mul(t2[:], t1[:], 0.01)
            nc.vector.tensor_tensor(out=h2.blk(mi)[:], in0=t1[:], in1=t2[:], op=ALU.max)

        _matmul_block(nc, pp, l2fn, W[w1], h1, BPC, 0)
        out = BMat(sp, nc, 128, BPC, f"o{tag}")

        def add_b(mi, m0, m1, ps):
            nc.vector.tensor_scalar(out=out.blk(mi)[:], in0=ps[:],
                                    scalar1=W[bm].blk(0)[0:128, 2:3],
                                    scalar2=None, op0=ALU.add)

        _matmul_block(nc, pp, add_b, W[w2], h2, BPC, 0)
        return out

    cage_raw = mlp3(scode, "wnc", "wnc1", "wnc2", "bnc", "nc")
    cage = sp.tile([128, BPC], F32, tag="cage")
    nc.vector.tensor_scalar(out=cage[:], in0=cage_raw.blk(0)[:],
                            scalar1=W["tmpl"].blk(0)[:, 0:1], scalar2=None, op0=ALU.add)
    for b_ in range(BPC):
        nc.sync.dma_start(outv[0, b_].rearrange("(c u) -> c u", u=1), cage[:, b_:b_ + 1])

    mcode = BMat(sp, nc, 1024, BPC, "mcode")
    for ki in range(4):
        nc.vector.tensor_copy(mcode.blk(ki)[:], scode.blk(ki)[:])
        nc.vector.tensor_copy(mcode.blk(4 + ki)[:], tcode.blk(ki)[:])
    tcv = BMat(sp, nc, 1024, BPC, "tcv")
    _matmul_block(nc, pp, lrelu_into(tcv, W["bmg"], 0, "mg"), W["wmg"], mcode, BPC, 0)
    ncage_raw = mlp3(tcv, "wnd", "wnd1", "wnd2", "bnd", "nd")
    ncage = sp.tile([128, BPC], F32, tag="ncage")
    nc.vector.tensor_tensor(out=ncage[:], in0=ncage_raw.blk(0)[:], in1=cage[:], op=ALU.add)
    for b_ in range(BPC):
        nc.sync.dma_start(outv[1, b_].rearrange("(c u) -> c u", u=1), ncage[:, b_:b_ + 1])
    ctx.close()
    _split_waits(nc)
    return nc


_PROGRAMS = {}


def _get_program(key, builder, *a):
    if key not in _PROGRAMS:
        _PROGRAMS[key] = builder(*a)
    return _PROGRAMS[key]


def _np(a):
    return np.asarray(a, dtype=np.float32)


def kernel(source_shape, target_shape, params):
    source_shape = _np(source_shape)
    target_shape = _np(target_shape)
    core_ids = list(range(NCORES))
    clouds = {"s": np.ascontiguousarray(np.transpose(source_shape, (0, 2, 1))),
              "t": np.ascontiguousarray(np.transpose(target_shape, (0, 2, 1)))}
    feats = {"s": None, "t": None}
    codes = {"s": [], "t": []}

    for lvl in range(3):
        S = NPOINTS[lvl]
        cin, c1, c2, c3 = CH[lvl]
        K1 = 16 * ((cin + 15) // 16)
        COLS = S * NSAMPLE
        gin = np.zeros((NCORES, NRUNS, K1, COLS), np.float32)
        for ci, cl in enumerate(("s", "t")):
            xyz = clouds[cl]
            fts = feats[cl]
            idx = _fps(xyz, S)
            new_xyz = np.take_along_axis(xyz, idx[..., None], axis=1)
            grp = _group(xyz, new_xyz, RADII[lvl], NSAMPLE)
            gx = np.take_along_axis(xyz, grp.reshape(B, -1)[..., None], axis=1
                                    ).reshape(B, S, NSAMPLE, 3)
            gx = gx - new_xyz[:, :, None, :]
            rows = np.transpose(gx, (0, 3, 1, 2)).reshape(B, 3, COLS)
            if fts is not None:
                gf = np.take_along_axis(fts, grp.reshape(B, 1, -1), axis=2
                                        ).reshape(B, fts.shape[1], COLS)
            for core in range(NCORES):
                for b_ in range(BPC):
                    gb, run = core * BPC + b_, ci * BPC + b_
                    gin[core, run, 0:3] = rows[gb]
                    if fts is not None:
                        gin[core, run, 3:3 + fts.shape[1]] = gf[gb]
            clouds[cl] = new_xyz

        ws = [_np(w) for w in params["sa"][lvl]["ws"]]
        bs = [_np(b) for b in params["sa"][lvl]["bs"]]
        w1h = np.zeros((K1, c1), np.float32)
        w1h[0:cin] = ws[0].T
        pwh = np.zeros((c3, 256), np.float32)
        pwh[:, 0:170] = _np(params["post"][lvl]["w"]).T
        pbh = np.zeros((256, 1), np.float32)
        pbh[0:170, 0] = _np(params["post"][lvl]["b"])
        shared = {"w1": w1h, "w2": np.ascontiguousarray(ws[1].T),
                  "w3": np.ascontiguousarray(ws[2].T), "pw": pwh,
                  "b1": bs[0][:, None].copy(), "b2": bs[1][:, None].copy(),
                  "b3": bs[2][:, None].copy(), "pb": pbh}
        prog = _get_program(("enc", lvl), _build_enc_program, lvl)
        res = run_bass_kernel_spmd(
            prog, [dict(shared, gin=gin[c]) for c in range(NCORES)], core_ids)
        fe = np.stack([res.results[c]["feats"] for c in range(NCORES)])
        cd = np.stack([res.results[c]["code"] for c in range(NCORES)])
        for ci, cl in enumerate(("s", "t")):
            feats[cl] = np.concatenate(
                [fe[c, ci * BPC:(ci + 1) * BPC] for c in range(NCORES)], 0)
            cc = np.concatenate(
                [cd[c, ci * BPC:(ci + 1) * BPC] for c in range(NCORES)], 0)
            codes[cl].append(cc[:, 0:170])

    s_code = np.concatenate(codes["s"], axis=1)
    t_code = np.concatenate(codes["t"], axis=1)

    def padw(wT, K, M):
        o = np.zeros((K, M), np.float32)
        o[: wT.shape[0], : wT.shape[1]] = wT
        return o

    ncl, ndl = params["nc"], params["nd"]
    mw, mb = params["merger"]
    bnc = np.zeros((512, 3), np.float32)
    bnc[0:512, 0] = _np(ncl[0][1]); bnc[0:256, 1] = _np(ncl[1][1])
    bnc[0:126, 2] = _np(ncl[2][1])
    bnd = np.zeros((512, 3), np.float32)
    bnd[0:512, 0] = _np(ndl[0][1]); bnd[0:256, 1] = _np(ndl[1][1])
    bnd[0:126, 2] = _np(ndl[2][1])
    tmpl = np.zeros((128, 1), np.float32)
    tmpl[0:126, 0] = _np(params["template"]).reshape(-1)
    # merger weight rows must match device mcode layout: s at 0:510, t at 512:1022
    mwT = _np(mw).T  # [1020, 1020]
    wmg = np.zeros((1024, 1024), np.float32)
    wmg[0:510, 0:1020] = mwT[0:510]
    wmg[512:1022, 0:1020] = mwT[510:1020]
    headw = {"wnc": padw(_np(ncl[0][0]).T, 512, 512),
             "wnc1": padw(_np(ncl[1][0]).T, 512, 256),
             "wnc2": padw(_np(ncl[2][0]).T, 256, 128),
             "wnd": padw(_np(ndl[0][0]).T, 1024, 512),
             "wnd1": padw(_np(ndl[1][0]).T, 512, 256),
             "wnd2": padw(_np(ndl[2][0]).T, 256, 128),
             "wmg": wmg, "bnc": bnc, "bnd": bnd,
             "bmg": padw(_np(mb)[:, None], 1024, 1), "tmpl": tmpl}
    codes_in = np.zeros((NCORES, NRUNS, 512), np.float32)
    for core in range(NCORES):
        for b_ in range(BPC):
            gb = core * BPC + b_
            codes_in[core, b_, 0:510] = s_code[gb]
            codes_in[core, BPC + b_, 0:510] = t_code[gb]
    prog = _get_program(("head",), _build_head_program)
    res = run_bass_kernel_spmd(
        prog, [dict(headw, codes=codes_in[c]) for c in range(NCORES)], core_ids)
    ov = np.stack([res.results[c]["outv"] for c in range(NCORES)])
    cage = np.concatenate([ov[c, 0] for c in range(NCORES)], 0)[:, 0:126]
    new_cage = np.concatenate([ov[c, 1] for c in range(NCORES)], 0)[:, 0:126]
    return cage.reshape(B, 3, P), new_cage.reshape(B, 3, P)
